# revision 1
# baseline (speedup 1.0000x reference)
"""Trainium2 Bass kernel for the NCE cosine-similarity loss.

Problem: x [65536, 1024] f32 viewed as 1024 batches x 64 rows (1 orig, 8 pos,
55 neg). Per batch: cos(orig,pos_i) and cos(pos_i,neg_j), logits/0.1,
loss = logsumexp([cp, cn_*]) - cp, mean over all (batch, pos).

Strategy (8 NeuronCores, data-parallel over batches, 128 batches/core):
 - Two batches share one 128-partition tile ("pair"): rows on partitions.
 - DMA cast-load fp32->bf16 (SWDGE), then one xbar DMA-transpose per group of
   8 pairs gives [d-chunk, row] layout.
 - Per pair: 64x64-per-batch Gram (as one 128x128 via 8 accumulating bf16
   matmuls over the 8 d-chunks of 128). Diagonal -> norms. inv = sqrt(10)/norm
   (folds the 1/tau=10 logit scale into both cosine normalizations).
 - Column scale (with excluded pos-pos columns zeroed -> exp()=1, subtract 8
   later) via a K=1 broadcast matmul + DVE multiply; row scale fused into
   ScalarE Exp activation which also accumulates the per-row sum.
 - loss row p: log(sum - 8) - l0. Per-core partial sums [128, 4] go to HBM;
   host combines the 8 cores and divides by 8192.
"""

import sys

if "/opt/trn_rl_repo" not in sys.path:
    sys.path.insert(0, "/opt/trn_rl_repo")

import numpy as np

N_CORES = 8
ROWS_PER_CORE = 8192          # 128 batches x 64 rows
D = 1024
N_GROUPS = 8                  # dma groups of 8 pairs per core
N_QUADS = 16                  # quads of 4 pairs per core
N_PAIRS = 64                  # 2 batches per pair

_CACHE = {}


def _build(repeat=1, loop_n=0, stage=8, dma_once=False):
    import concourse.bacc as bacc
    import concourse.mybir as mybir
    import concourse.tile as tile

    dt = mybir.dt
    AF = mybir.ActivationFunctionType
    ALU = mybir.AluOpType

    nc = bacc.Bacc("TRN2", target_bir_lowering=False, debug=False, num_devices=N_CORES)
    x = nc.dram_tensor("x", [ROWS_PER_CORE, D], dt.float32, kind="ExternalInput")
    identb_d = nc.dram_tensor("identb", [128, 128], dt.bfloat16, kind="ExternalInput")
    mask_d = nc.dram_tensor("mask", [4, 128], dt.bfloat16, kind="ExternalInput")
    sel_d = nc.dram_tensor("sel", [4, 512], dt.bfloat16, kind="ExternalInput")
    neg8_d = nc.dram_tensor("neg8", [128, 1], dt.float32, kind="ExternalInput")
    out_d = nc.dram_tensor("out", [128, 4], dt.float32, kind="ExternalOutput")

    # x rows (g j p) d: group g, pair-in-group j, partition p
    xg = x.rearrange("(g j p) d -> g p j d", g=N_GROUPS, j=8, p=128)

    with tile.TileContext(nc) as tc:
        from contextlib import ExitStack

        with ExitStack() as ctx:
            cpool = ctx.enter_context(tc.tile_pool(name="consts", bufs=1))
            rowp = ctx.enter_context(tc.tile_pool(name="row", bufs=3))
            tp = ctx.enter_context(tc.tile_pool(name="tgrp", bufs=8))
            tpsp = ctx.enter_context(tc.tile_pool(name="tps", bufs=4, space="PSUM"))
            gramp = ctx.enter_context(tc.tile_pool(name="gram", bufs=2, space="PSUM"))
            bcpsp = ctx.enter_context(tc.tile_pool(name="bcps", bufs=2, space="PSUM"))
            gsbp = ctx.enter_context(tc.tile_pool(name="gsb", bufs=10))
            sb = ctx.enter_context(tc.tile_pool(name="sb", bufs=2))
            t2p = ctx.enter_context(tc.tile_pool(name="t2", bufs=3))
            stg = ctx.enter_context(tc.tile_pool(name="stg", bufs=1))

            identb = cpool.tile([128, 128], dt.bfloat16)
            nc.sync.dma_start(out=identb[:], in_=identb_d[:])
            maskc = cpool.tile([4, 128], dt.bfloat16)
            nc.sync.dma_start(out=maskc[:], in_=mask_d[:])
            selc = cpool.tile([4, 512], dt.bfloat16)
            nc.sync.dma_start(out=selc[:], in_=sel_d[:])
            neg8c = cpool.tile([128, 1], dt.float32)
            nc.sync.dma_start(out=neg8c[:], in_=neg8_d[:])

            s_stage = stg.tile([128, 2, 64], dt.float32, tag="s_stage")
            l0_stage = stg.tile([128, 2, 64], dt.float32, tag="l0_stage")

            from contextlib import nullcontext

            def phase_a(q, row_tiles):
                """loads, transposes, grams, diag sums for quad q"""
                if q % 2 == 0 and (not dma_once or not row_tiles):
                    G = 0 if dma_once else q // 2
                    row = rowp.tile([128, 8, D], dt.bfloat16, tag="row")
                    nc.gpsimd.dma_start(out=row[:], in_=xg[G])
                    row_tiles[G] = row
                row = row_tiles[0 if dma_once else q // 2]

                n2q = sb.tile([128, 4], dt.float32, tag="n2q")
                grams = []
                t_list = []
                for jj in range(4):
                    j_in_g = (q % 2) * 4 + jj
                    g_abs = 4 * q + jj
                    tps = tpsp.tile([128, 8, 128], dt.bfloat16, tag="tps")
                    for c in range(8):
                        nc.tensor.transpose(
                            tps[:, c, :],
                            row[:, j_in_g, c * 128 : (c + 1) * 128],
                            identb[:],
                        )
                    t = tp.tile([128, 8, 128], dt.bfloat16, tag="t")
                    if g_abs % 2 == 0:
                        nc.vector.tensor_copy(t.rearrange("p a b -> p (a b)"),
                                              tps.rearrange("p a b -> p (a b)"))
                    else:
                        nc.scalar.copy(t.rearrange("p a b -> p (a b)"),
                                       tps.rearrange("p a b -> p (a b)"))
                    t_list.append(t)
                if stage < 2:
                    return n2q, grams
                for jj in range(4):
                    g_abs = 4 * q + jj
                    t = t_list[jj]
                    gps = gramp.tile([128, 128], dt.float32, tag="gram")
                    for c in range(8):
                        nc.tensor.matmul(
                            gps[:],
                            t[:, c, :],
                            t[:, c, :],
                            start=(c == 0),
                            stop=(c == 7),
                        )
                    gsb = gsbp.tile([128, 128], dt.bfloat16, tag="gsb")
                    if g_abs % 2 == 0:
                        nc.scalar.copy(gsb[:], gps[:])
                    else:
                        nc.vector.tensor_copy(gsb[:], gps[:])
                    scr = sb.tile([128, 128], dt.bfloat16, tag="scr")
                    nc.vector.tensor_mul(scr[:], gsb[:], identb[:])
                    nc.vector.reduce_sum(
                        n2q[:, jj : jj + 1], scr[:], axis=mybir.AxisListType.X
                    )
                    grams.append(gsb)
                return n2q, grams

            def phase_b(q, n2q, grams):
                """normalization + exp/log-sum staging for quad q"""
                if stage < 4:
                    return
                # inv = sqrt(10/n2) = exp(-0.5*ln(0.1*n2)): Ln/Exp share an
                # ACT table set with Copy, avoiding per-quad table reloads
                lnn2 = sb.tile([128, 4], dt.float32, tag="lnn2")
                nc.scalar.activation(lnn2[:], n2q[:], AF.Ln, scale=0.1)
                invq = sb.tile([128, 4], dt.float32, tag="invq")
                nc.scalar.activation(invq[:], lnn2[:], AF.Exp, scale=-0.5)

                if stage < 5:
                    return
                invb = sb.tile([128, 4], dt.bfloat16, tag="invb")
                nc.vector.tensor_copy(invb[:], invq[:])
                itps = bcpsp.tile([4, 128], dt.float32, tag="bcps")
                nc.tensor.matmul(itps[:], invb[:], identb[:])
                itsb = sb.tile([4, 128], dt.bfloat16, tag="itsb")
                nc.vector.tensor_mul(itsb[:], itps[:], maskc[:])

                if stage < 6:
                    return
                t2q = t2p.tile([128, 4, 128], dt.float32, tag="t2")
                for jj in range(4):
                    bcps = bcpsp.tile([128, 128], dt.float32, tag="bcps")
                    nc.tensor.matmul(
                        bcps[:], selc[:, jj * 128 : (jj + 1) * 128], itsb[:]
                    )
                    # fully-scaled logits: (G * inv[p]) * (inv_masked[j])
                    nc.vector.scalar_tensor_tensor(
                        t2q[:, jj, :],
                        grams[jj][:],
                        invq[:, jj : jj + 1],
                        bcps[:],
                        op0=ALU.mult,
                        op1=ALU.mult,
                    )
                if stage < 7:
                    return
                escr = sb.tile([128, 4, 128], dt.float32, tag="escr")
                nc.scalar.activation(
                    escr.rearrange("p a b -> p (a b)"),
                    t2q.rearrange("p a b -> p (a b)"),
                    AF.Exp,
                )
                nc.vector.reduce_sum(
                    s_stage[:, :, 4 * q : 4 * q + 4].rearrange("p h g -> p g h"),
                    escr.rearrange("p a (c h) -> p (a c) h", h=64),
                    axis=mybir.AxisListType.X,
                )
                nc.vector.tensor_copy(
                    l0_stage[:, :, 4 * q : 4 * q + 4].rearrange("p h g -> p g h"),
                    t2q.rearrange("p a (c h) -> p a c", h=64)
                    if False
                    else t2q.rearrange("p a b -> p (a b)")[:, 0:512:64].rearrange(
                        "p (g h) -> p g h", h=2
                    ),
                )

            loop_cm = tc.For_i(0, loop_n, 1) if loop_n else nullcontext()
            with loop_cm:
                row_tiles = {}
                pending = None
                for q in range(N_QUADS * repeat):
                    q = q % N_QUADS
                    state = phase_a(q, row_tiles)
                    if pending is not None:
                        phase_b(pending[0], pending[1], pending[2])
                    pending = (q, *state)
                if pending is not None:
                    phase_b(pending[0], pending[1], pending[2])

            final = stg.tile([128, 4], dt.float32, tag="final")
            if stage >= 7:
                lnout = stg.tile([128, 2, 64], dt.float32, tag="lnout")
                nc.scalar.activation(
                    lnout.rearrange("p a b -> p (a b)"),
                    s_stage.rearrange("p a b -> p (a b)"),
                    AF.Ln,
                    bias=neg8c[:],
                )
                nc.vector.reduce_sum(
                    final[:, 0:2], lnout[:], axis=mybir.AxisListType.X
                )
                nc.vector.reduce_sum(
                    final[:, 2:4], l0_stage[:], axis=mybir.AxisListType.X
                )
            else:
                nc.vector.memset(final[:], 0.0)
                nc.vector.memset(s_stage.rearrange("p a b -> p (a b)"), 0.0)
                nc.vector.memset(l0_stage.rearrange("p a b -> p (a b)"), 0.0)
            nc.gpsimd.dma_start(out=out_d[:], in_=final[:])

    nc.compile()
    return nc


def _consts():
    import ml_dtypes

    bf = ml_dtypes.bfloat16
    ident = np.eye(128, dtype=np.float32)  # unused on device now
    mask = np.ones((4, 128), dtype=bf)
    mask[:, 1:9] = 0.0
    mask[:, 65:73] = 0.0
    sel = np.zeros((4, 512), dtype=bf)
    for jj in range(4):
        sel[jj, jj * 128 : (jj + 1) * 128] = 1.0
    neg8 = np.full((128, 1), -8.0, dtype=np.float32)
    identb = np.eye(128, dtype=bf)
    return ident, mask, sel, neg8, identb


def kernel(x, labels=None, **_unused):
    from concourse.bass_utils import run_bass_kernel_spmd

    x = np.ascontiguousarray(np.asarray(x, dtype=np.float32))
    assert x.shape == (N_CORES * ROWS_PER_CORE, D), x.shape

    if "nc" not in _CACHE:
        _CACHE["nc"] = _build()
    nc = _CACHE["nc"]

    ident, mask, sel, neg8, identb = _consts()
    in_maps = [
        {
            "x": x[i * ROWS_PER_CORE : (i + 1) * ROWS_PER_CORE],
            "mask": mask,
            "sel": sel,
            "neg8": neg8,
            "identb": identb,
        }
        for i in range(N_CORES)
    ]
    res = run_bass_kernel_spmd(nc, in_maps, list(range(N_CORES)))

    total = 0.0
    for r in res.results:
        o = r["out"].astype(np.float64)
        # valid rows: pos rows of batch A (partitions 1..8, half A) and of
        # batch B (partitions 65..72, half B)
        total += o[1:9, 0].sum() - o[1:9, 2].sum()
        total += o[65:73, 1].sum() - o[65:73, 3].sum()
    loss = total / (1024 * 8)
    return np.array(loss, dtype=np.float32)



# revision 6
# speedup vs baseline: 1.1188x; 1.1188x over previous
"""Trainium2 Bass kernel for the NCE cosine-similarity loss.

Problem: x [65536, 1024] f32 viewed as 1024 batches x 64 rows (1 orig, 8 pos,
55 neg). Per batch: cos(orig,pos_i) and cos(pos_i,neg_j), logits/0.1,
loss = logsumexp([cp, cn_*]) - cp, mean over all (batch, pos).

Strategy (8 NeuronCores, data-parallel over batches, 128 batches/core):
 - Host staging: shard over cores, pre-cast to bf16, pre-TRANSPOSE so the
   contraction dim d lands on SBUF partitions (kills all PE transposes that
   dominated the previous version), and reorder each pair's 128 rows to
   [posA(8) posB(8) origA origB negA(55) negB(55)].
 - Per pair (2 batches = 128 rows): full 128x128 Gram via 8 accumulating
   bf16 matmuls (K=128 d-chunks, stream N=128). Diagonal = squared norms
   (one fused DVE tensor_tensor_reduce vs identity). Pos rows 0..15 of each
   pair's Gram are the only cosines needed.
 - Per group of 8 pairs, everything else is batched on 128 partitions:
   inv = sqrt(10)/n (folds 1/tau; ACT Ln+Exp), a masked scale matrix
   bcig[16j+i, r] = inv_j[i]*inv_j[r]*mask (built with 3 tiny PE matmuls),
   one DVE multiply -> logits, one ACT Exp with accum_out -> row sums
   (excluded logits are scaled to 0 so each contributes exp(0)=1; the final
   log subtracts the 72 ones via its bias), one fused reduce -> l0.
 - loss rows stage into [128, 8]; final Ln(sum-72)+accum and a reduce give
   [128, 2] per core; host combines 8 cores and divides by 8192.
"""

import sys

if "/opt/trn_rl_repo" not in sys.path:
    sys.path.insert(0, "/opt/trn_rl_repo")

import numpy as np

N_CORES = 8
ROWS_PER_CORE = 8192          # 128 batches x 64 rows
D = 1024
N_GROUPS = 8                  # groups of 8 pairs per core
N_PAIRS = 64                  # 2 batches per pair
NB = 64                       # rows per batch
NPOS = 8

_CACHE = {}


def _build(loop_n=0, fp8=False):
    import concourse.bacc as bacc
    import concourse.mybir as mybir
    import concourse.tile as tile

    dt = mybir.dt
    AF = mybir.ActivationFunctionType
    ALU = mybir.AluOpType
    xdt = dt.float8e4 if fp8 else dt.bfloat16

    nc = bacc.Bacc("TRN2", target_bir_lowering=False, debug=False, num_devices=N_CORES)
    x = nc.dram_tensor("x", [N_GROUPS, 128, 8 * D], xdt, kind="ExternalInput")
    identb_d = nc.dram_tensor("identb", [128, 128], dt.bfloat16, kind="ExternalInput")
    maskab_d = nc.dram_tensor("maskab", [16, 128], dt.bfloat16, kind="ExternalInput")
    pattb_d = nc.dram_tensor("pattb", [16, 256], dt.bfloat16, kind="ExternalInput")
    l0mask_d = nc.dram_tensor("l0mask", [128, 2], dt.bfloat16, kind="ExternalInput")
    neg72_d = nc.dram_tensor("neg72", [128, 1], dt.float32, kind="ExternalInput")
    out_d = nc.dram_tensor("out", [128, 2], dt.float32, kind="ExternalOutput")

    with tile.TileContext(nc) as tc:
        from contextlib import ExitStack, nullcontext

        with ExitStack() as ctx:
            cpool = ctx.enter_context(tc.tile_pool(name="consts", bufs=1))
            rowp = ctx.enter_context(tc.tile_pool(name="row", bufs=2))
            gramp = ctx.enter_context(tc.tile_pool(name="gram", bufs=4, space="PSUM"))
            bcpsp = ctx.enter_context(tc.tile_pool(name="bcps", bufs=2, space="PSUM"))
            itpsp = ctx.enter_context(tc.tile_pool(name="itps", bufs=1, space="PSUM"))
            sb = ctx.enter_context(tc.tile_pool(name="sb", bufs=2))
            scrp = ctx.enter_context(tc.tile_pool(name="scr", bufs=2))
            stg = ctx.enter_context(tc.tile_pool(name="stg", bufs=1))

            identb = cpool.tile([128, 128], dt.bfloat16)
            nc.sync.dma_start(out=identb[:], in_=identb_d[:])
            maskab = cpool.tile([16, 128], dt.bfloat16)
            nc.sync.dma_start(out=maskab[:], in_=maskab_d[:])
            pattb = cpool.tile([16, 256], dt.bfloat16)
            nc.sync.dma_start(out=pattb[:], in_=pattb_d[:])
            l0mask = cpool.tile([128, 2], dt.bfloat16)
            nc.sync.dma_start(out=l0mask[:], in_=l0mask_d[:])
            neg72 = cpool.tile([128, 1], dt.float32)
            nc.sync.dma_start(out=neg72[:], in_=neg72_d[:])

            sums_stage = stg.tile([128, 2 * N_GROUPS], dt.float32, tag="sums")
            l0_stage = stg.tile([128, 2 * N_GROUPS], dt.float32, tag="l0s")

            def group(g):
                rt = rowp.tile([128, 8, 8, 128], xdt, tag="rt")
                nc.sync.dma_start(out=rt[:], in_=x[g])

                n2g = sb.tile([128, 8], dt.float32, tag="n2g")
                # posG[32*(j%4)+u, 128*(j//4)+r] = G_j[u, r]; only u<16 (pos
                # rows) are used downstream, u in 16..32 ride along so every
                # partition start stays 32-aligned (HW AP constraint).
                posG = sb.tile([128, 2, 128], dt.bfloat16, tag="posG")
                for j in range(8):
                    gps = gramp.tile([128, 128], dt.float32, tag="gram")
                    for c in range(8):
                        nc.tensor.matmul(
                            gps[:],
                            rt[:, j, c, :],
                            rt[:, j, c, :],
                            start=(c == 0),
                            stop=(c == 7),
                        )
                    dscr = scrp.tile([128, 128], dt.float16, tag="dscr")
                    nc.vector.tensor_mul(dscr[:], gps[:], identb[:])
                    nc.vector.reduce_sum(
                        n2g[:, j : j + 1], dscr[:], axis=mybir.AxisListType.X
                    )
                    jp, e = j % 4, j // 4
                    nc.scalar.copy(posG[32 * jp : 32 * jp + 32, e, :], gps[0:32, :])

                # inv = sqrt(10)/n = exp(-0.5*ln(0.1*n2)); Ln/Exp share a table set
                lnn2 = sb.tile([128, 8], dt.float32, tag="lnn2")
                nc.scalar.activation(lnn2[:], n2g[:], AF.Ln, scale=0.1)
                invf = sb.tile([128, 8], dt.float32, tag="invf")
                nc.scalar.activation(invf[:], lnn2[:], AF.Exp, scale=-0.5)
                invb2 = sb.tile([128, 16], dt.bfloat16, tag="invb2")
                nc.vector.tensor_copy(invb2[:, 0::2], invf[:])
                nc.vector.tensor_copy(invb2[:, 1::2], invf[:])

                # itps2[2p+h, r] = inv_p[r]; mi = masked rows; misb = pos-scale rows
                itps2 = itpsp.tile([16, 128], dt.float32, tag="itps2")
                nc.tensor.matmul(itps2[:], invb2[:], identb[:], start=True, stop=True)
                mi = sb.tile([16, 128], dt.bfloat16, tag="mi")
                nc.vector.tensor_mul(mi[:], itps2[:], maskab[:])
                isb = sb.tile([16, 16], dt.bfloat16, tag="isb")
                nc.scalar.copy(isb[:], itps2[:, 0:16])
                misb = sb.tile([16, 256], dt.bfloat16, tag="misb")
                nc.vector.tensor_mul(
                    misb.rearrange("p (a b) -> p a b", a=16),
                    isb.rearrange("p (o b) -> p o b", o=1).broadcast_to([16, 16, 16]),
                    pattb.rearrange("p (a b) -> p a b", a=16),
                )

                # bcig[32*(j%4)+u, j//4, r] = inv_j[u]*inv_j[r]*mask_{h(u)}[r]
                # for u<16, exactly 0 for u>=16 (garbage rows -> exp(0)=1)
                bcps = bcpsp.tile([128, 2, 128], dt.float32, tag="bcps")
                for j in range(8):
                    jp, e = j % 4, j // 4
                    tp = {"tile_position": (0, 96)} if jp == 3 else {}
                    nc.tensor.matmul(
                        bcps[32 * jp : 32 * jp + 32, e, :],
                        misb[:, 32 * j : 32 * j + 32],
                        mi[:],
                        start=True, stop=True, **tp,
                    )
                bcig = sb.tile([128, 2, 128], dt.bfloat16, tag="bcig")
                nc.scalar.copy(bcig[:], bcps[:])

                # fully-scaled logits; excluded entries are exactly 0
                t2g = sb.tile([128, 2, 128], dt.float16, tag="t2g")
                nc.vector.tensor_mul(t2g[:], posG[:], bcig[:])

                for e in range(2):
                    escr = scrp.tile([128, 128], dt.float16, tag="escr")
                    nc.scalar.activation(
                        escr[:], t2g[:, e, :], AF.Exp,
                        accum_out=sums_stage[:, 2 * g + e : 2 * g + e + 1],
                    )
                    l0scr = scrp.tile([128, 2], dt.float16, tag="l0scr")
                    nc.vector.tensor_mul(l0scr[:], t2g[:, e, 16:18], l0mask[:])
                    nc.vector.reduce_sum(
                        l0_stage[:, 2 * g + e : 2 * g + e + 1], l0scr[:],
                        axis=mybir.AxisListType.X,
                    )

            loop_cm = tc.For_i(0, loop_n, 1) if loop_n else nullcontext()
            with loop_cm:
                for g in range(N_GROUPS):
                    group(g)

            final = stg.tile([128, 2], dt.float32, tag="final")
            lnscr = stg.tile([128, 2 * N_GROUPS], dt.float32, tag="lnscr")
            nc.scalar.activation(
                lnscr[:], sums_stage[:], AF.Ln, bias=neg72[:],
                accum_out=final[:, 0:1],
            )
            nc.vector.reduce_sum(final[:, 1:2], l0_stage[:], axis=mybir.AxisListType.X)
            nc.gpsimd.dma_start(out=out_d[:], in_=final[:])

    nc.compile()
    return nc


def _consts():
    import ml_dtypes

    bf = ml_dtypes.bfloat16
    identb = np.eye(128, dtype=bf)
    # row r = 2p+h: mask_h over staged cols [pos16, origA, origB, negA55, negB55]
    maskab = np.zeros((16, 128), dtype=np.float32)
    maskab[0::2, 16] = 1.0
    maskab[0::2, 18:73] = 1.0
    maskab[1::2, 17] = 1.0
    maskab[1::2, 73:128] = 1.0
    # pattb[k, 32j+m]: nonzero iff m<16 and k == 2j + (0 if m<8 else 1)
    pattb = np.zeros((16, 256), dtype=np.float32)
    for j in range(8):
        for m in range(16):
            k = 2 * j + (0 if m < 8 else 1)
            pattb[k, 32 * j + m] = 1.0
    # partition p = 32*q' + u: valid pos rows are u<16; cp col is 16+h(u)
    l0mask = np.zeros((128, 2), dtype=np.float32)
    for p in range(128):
        u = p % 32
        if u < 16:
            l0mask[p, 0 if u < 8 else 1] = 1.0
    neg72 = np.full((128, 1), -72.0, dtype=np.float32)
    return identb, maskab.astype(bf), pattb.astype(bf), l0mask.astype(bf), neg72


_PERM_AB = np.array([0] * 8 + [1] * 8 + [0, 1] + [0] * 55 + [1] * 55)
_PERM_ROW = np.array(
    list(range(1, 9)) + list(range(1, 9)) + [0, 0]
    + list(range(9, 64)) + list(range(9, 64))
)


def _stage_core(xc, xdt):
    """xc: [8192, 1024] fp32 -> staged [8, 128, 8192] in xdt."""
    xb = xc.astype(xdt)                      # cast first (cheaper moves)
    xp = xb.reshape(64, 2, 64, D)[:, _PERM_AB, _PERM_ROW, :]   # [pair, r, d]
    xt = xp.reshape(64, 128, 8, 128).transpose(0, 3, 2, 1)     # [pair, dp, c, r]
    xt = xt.reshape(8, 8, 128, 8 * 128).transpose(0, 2, 1, 3)  # [g, dp, j, c*r]
    return np.ascontiguousarray(xt.reshape(8, 128, 8 * D))


def kernel(x, labels=None, fp8=False, **_unused):
    from concourse.bass_utils import run_bass_kernel_spmd
    import concourse.mybir as mybir

    x = np.ascontiguousarray(np.asarray(x, dtype=np.float32))
    assert x.shape == (N_CORES * ROWS_PER_CORE, D), x.shape

    key = "fp8" if fp8 else "bf16"
    if key not in _CACHE:
        _CACHE[key] = _build(fp8=fp8)
    nc = _CACHE[key]

    xdt = mybir.dt.np(mybir.dt.float8e4) if fp8 else None
    if xdt is None:
        import ml_dtypes

        xdt = ml_dtypes.bfloat16

    identb, maskab, pattb, l0mask, neg72 = _consts()
    in_maps = [
        {
            "x": _stage_core(x[i * ROWS_PER_CORE : (i + 1) * ROWS_PER_CORE], xdt),
            "identb": identb,
            "maskab": maskab,
            "pattb": pattb,
            "l0mask": l0mask,
            "neg72": neg72,
        }
        for i in range(N_CORES)
    ]
    res = run_bass_kernel_spmd(nc, in_maps, list(range(N_CORES)))

    valid = (np.arange(128) % 32) < 16
    total = 0.0
    for r in res.results:
        o = r["out"].astype(np.float64)
        total += o[valid, 0].sum() - o[valid, 1].sum()
    loss = total / (1024.0 * NPOS)
    return np.array(loss, dtype=np.float32)
